# revision 7
# baseline (speedup 1.0000x reference)
"""Bass/Trainium2 kernel for nn_Attention_Layer (B=8, N=4096, D=128).

Sharding: data-parallel over batch B across the 8 NeuronCores (one batch
element per core); the 128x128 Q/K/V weights are replicated.

Per-core algorithm (X = att_input[b], [4096, 128] fp32):
  1. Setup: X loaded via 4 parallel DMA queues.  PE-transposes X
     (quad-batched into PSUM); ACT evacuates+converts to fp16 xt.
     K/Q projections per 512-chunk (fp16 matmuls) evacuated to fp16
     kt/qt on DVE.  V = Xt.T @ WvT (fp16) evacuated to bf16 vext pairs
     (ones column accumulates the softmax denominator), alternating
     ACT/DVE.
  2. Main loop over 128 groups (2 k-tiles x 512 q):
       iteration g emits: S(g+1) [2 fp16 512-row matmuls, 216 ns each],
       then PV(g-1) [8 bf16 129-row matmuls, 57 ns back-to-back], then
       exp(g) [one 1024-wide ACT instruction, ~1010 ns].
     S runs one group ahead so it always completes during exp(g); the
     ACT engine never waits.  PE work/group (~950 ns) < exp (~1010 ns).
  3. Per chunk: DVE-drain O, reciprocal of ones-column sums, normalize,
     one 256KB DMA out.

dtypes: fp16 for X^T/W/Q/K (5x error margin vs bf16), bf16 for P and V
(P needs bf16 range: unnormalized exp reaches ~3.6e9), fp32 PSUM accum.
softmax max-subtraction is skipped: scores have std ~3.8, max ~22.
PSUM: S groups 2x2 banks (double buffered) + O 4 banks (129 fp32 each).
"""

import sys

if "/opt/trn_rl_repo" not in sys.path:
    sys.path.insert(0, "/opt/trn_rl_repo")

import numpy as np

import concourse.bass as bass
import concourse.mybir as mybir
import concourse.tile as tile
from concourse import bacc
from concourse.bass_utils import run_bass_kernel_spmd
from concourse.masks import make_identity

B, N, D = 8, 4096, 128
P = 128                 # partitions / tile edge
NT = N // P             # 32 n-tiles (also k-tiles)
QC = 512                # q-chunk width (max moving free dim)
NQC = N // QC           # 8 q-chunks
QT = QC // P            # 4 q-tiles per chunk
TPG = 2                 # k-tiles per exp group (exp width = TPG*512)
NG = NT // TPG          # groups per chunk (16)
NGT = NQC * NG          # total groups (128)
F32 = mybir.dt.float32
FP16 = mybir.dt.float16
BF16 = mybir.dt.bfloat16
EXPF = mybir.ActivationFunctionType.Exp

_compiled = None


def _build():
    nc = bacc.Bacc("TRN2", target_bir_lowering=False, debug=False)
    x_d = nc.dram_tensor("x", [N, D], F32, kind="ExternalInput")
    wq_d = nc.dram_tensor("wq", [D, D], F32, kind="ExternalInput")
    wk_d = nc.dram_tensor("wk", [D, D], F32, kind="ExternalInput")
    wv_d = nc.dram_tensor("wv", [D, D], F32, kind="ExternalInput")
    out_d = nc.dram_tensor("out", [N, D], F32, kind="ExternalOutput")
    out_r = out_d.rearrange("(t p) d -> p t d", p=P)

    with tile.TileContext(nc) as tc:
        with (
            tc.tile_pool(name="singles", bufs=1) as singles,
            tc.tile_pool(name="stage", bufs=2) as stage,
            tc.tile_pool(name="ptp", bufs=4) as ptp,
            tc.tile_pool(name="outp", bufs=2) as outp,
        ):
            ident = singles.tile([P, P], F32)
            make_identity(nc, ident)
            zbias = singles.tile([P, 1], F32)
            nc.vector.memset(zbias, 0.0)

            # preload the exp table while DMAs stream in
            scratch = singles.tile([P, 1], F32)
            nc.scalar.activation(scratch, zbias, EXPF, bias=zbias)

            # ---- load weights + X across 3 DMA queues; X group 0 heads the
            # sync queue so the transpose pipeline starts ASAP ----
            dma_engs = [nc.sync, nc.gpsimd, nc.scalar]
            xn = singles.tile([P, NT, D], F32)
            x_r = x_d.rearrange("(t p) d -> p t d", p=P)
            nc.sync.dma_start(
                out=xn[:, 0:QT, :], in_=x_r[:, 0:QT, :]
            )
            w_sb = {}
            for i, (name, wd) in enumerate(
                (("wq", wq_d), ("wk", wk_d), ("wv", wv_d))
            ):
                t = stage.tile([P, P], F32, tag="wload", name=f"{name}_nat")
                dma_engs[(i + 1) % 3].dma_start(out=t, in_=wd[:, :])
                w_sb[name] = t
            for g in range(1, NQC):
                dma_engs[g % 3].dma_start(
                    out=xn[:, QT * g : QT * (g + 1), :],
                    in_=x_r[:, QT * g : QT * (g + 1), :],
                )

            xt = singles.tile([P, NT, P], FP16)
            qt = [None] * NQC
            kt = [None] * NQC
            # vext pairs: [P, 2, P+1] bf16, ones in col P
            vps_sb = [
                singles.tile([P, 2, P + 1], BF16, name=f"vx{i}") for i in range(NT // 2)
            ]
            for i in range(NT // 2):
                nc.gpsimd.memset(vps_sb[i][:, :, P : P + 1], 1.0)

            # ---- setup phase (own PSUM pool, released before main loop) ----
            wmrhs = singles.tile([P, QC], F32)
            nc.vector.memset(wmrhs, 0.0)
            with tc.tile_pool(name="stage_ps", bufs=2, space="PSUM") as sps:
                # PE warmup: ~3.5us of dummy fp32 matmuls ramp the PE clock
                # to full speed while the X DMAs are still in flight
                for _ in range(2):
                    wm = sps.tile([P, QC], F32, tag="pps", bufs=3, name="warm_ps")
                    nc.tensor.matmul(wm, lhsT=ident, rhs=wmrhs, start=True, stop=True)
                # weight transposes -> [d, e] fp16
                wT = {}
                for name in ("wq", "wk", "wv"):
                    ps = sps.tile([P, P], F32, tag="wtps", bufs=1, name=f"{name}T_ps")
                    nc.tensor.transpose(ps, w_sb[name], ident)
                    t = singles.tile([P, P], FP16, name=f"{name}T")
                    nc.vector.tensor_copy(t, ps)
                    wT[name] = t

                def _proj(dst, w, nm, c):
                    pps = sps.tile([P, QC], F32, tag="pps", bufs=3, name="proj_ps")
                    nc.tensor.matmul(
                        pps,
                        lhsT=w,
                        rhs=xt[:, QT * c : QT * (c + 1), :],
                        start=True,
                        stop=True,
                    )
                    d_ = singles.tile([P, QC], FP16, tag=f"{nm}{c}", name=f"{nm}{c}")
                    nc.vector.tensor_copy(d_, pps)
                    dst[c] = d_

                # per 4-tile load group: quad transposes (ACT evacuates),
                # kt+qt projections (DVE evacuates), then the previous
                # group's V pairs (keeps the PE stream dense while this
                # group's X tiles are still in DMA flight)
                def _vpair(pair):
                    vps = sps.tile([P, 2, P], F32, tag="vps", name="v_ps")
                    nc.tensor.matmul(
                        vps[:, 0, :], lhsT=xt[:, 2 * pair, :], rhs=wT["wv"],
                        start=True, stop=True,
                    )
                    nc.tensor.matmul(
                        vps[:, 1, :], lhsT=xt[:, 2 * pair + 1, :], rhs=wT["wv"],
                        start=True, stop=True,
                    )
                    if pair % 2 == 0:
                        nc.scalar.copy(vps_sb[pair][:, :, 0:P], vps)
                    else:
                        nc.vector.tensor_copy(vps_sb[pair][:, :, 0:P], vps)

                for g in range(NQC):
                    tps = sps.tile([P, QT, P], F32, tag="tps", name="xt_ps")
                    for i in range(QT):
                        nc.tensor.transpose(tps[:, i, :], xn[:, QT * g + i, :], ident)
                    nc.scalar.copy(xt[:, QT * g : QT * (g + 1), :], tps)
                    _proj(kt, wT["wk"], "kt", g)
                    _proj(qt, wT["wq"], "qt", g)
                    if g > 0:
                        _vpair(2 * g - 2)
                        _vpair(2 * g - 1)
                _vpair(NT // 2 - 2)
                _vpair(NT // 2 - 1)

            # ---- main attention loop ----
            with (
                tc.tile_pool(name="spsum", bufs=2, space="PSUM") as spsum,
                tc.tile_pool(name="opsum", bufs=1, space="PSUM") as opsum,
            ):
                def S_group(gg):
                    c, g = divmod(gg, NG)
                    sg = spsum.tile([P, TPG, QC], F32, tag="sg", name="s_ps")
                    for i in range(TPG):
                        t = TPG * g + i
                        nc.tensor.matmul(
                            sg[:, i, :],
                            lhsT=kt[t // QT][:, (t % QT) * P : (t % QT + 1) * P],
                            rhs=qt[c],
                            start=True,
                            stop=True,
                        )
                    return sg

                o_ps = None

                def PV(gg, o_ps):
                    g = gg % NG
                    pt = pts[gg % 4]
                    for i in range(TPG):
                        tp = TPG * g + i
                        for j in range(QT):
                            nc.tensor.matmul(
                                o_ps[j],
                                lhsT=pt[:, i, j * P : (j + 1) * P],
                                rhs=vps_sb[tp // 2][:, tp % 2, :],
                                start=(tp == 0),
                                stop=(tp == NT - 1),
                                skip_group_check=True,
                            )

                def drain(c, last=False):
                    # On the last chunk ACT is idle: split the drain across
                    # ACT and DVE to shorten the tail.
                    oc = outp.tile([P, QT, P + 1], F32, tag="oc", name="oc")
                    for j in range(QT):
                        if last and j < 2:
                            nc.scalar.copy(oc[:, j, :], o_ps[j])
                        else:
                            nc.vector.tensor_copy(oc[:, j, :], o_ps[j])
                    ot = outp.tile([P, QT, P], F32, tag="ot", name="ot")
                    for j in range(QT):
                        rinv = outp.tile([P, 1], F32, tag="rinv", name="rinv")
                        nc.vector.reciprocal(rinv, oc[:, j, P : P + 1])
                        if last and j < 2:
                            nc.scalar.activation(
                                ot[:, j, :], oc[:, j, 0:P],
                                mybir.ActivationFunctionType.Copy,
                                bias=0.0, scale=rinv[:, 0:1],
                            )
                        else:
                            nc.vector.tensor_scalar_mul(
                                ot[:, j, :], oc[:, j, 0:P], rinv[:, 0:1]
                            )
                    for j in range(QT):
                        dma_engs[j % 3].dma_start(
                            out=out_r[:, QT * c + j, :], in_=ot[:, j, :]
                        )

                pts = [None] * 4
                sg_cur = S_group(0)
                for gg in range(NGT):
                    sg_next = S_group(gg + 1) if gg < NGT - 1 else None
                    if gg % NG == 1:
                        # first PV of a chunk: allocate fresh O accumulators
                        o_ps = [
                            opsum.tile([P, P + 1], F32, tag=f"o{j}", name=f"o{j}")
                            for j in range(QT)
                        ]
                    if gg > 0:
                        PV(gg - 1, o_ps)
                        if (gg - 1) % NG == NG - 1:
                            drain((gg - 1) // NG)
                    pt = ptp.tile([P, TPG, QC], BF16, tag="pt", name="pt")
                    nc.scalar.activation(pt, sg_cur, EXPF, bias=zbias)
                    pts[gg % 4] = pt
                    sg_cur = sg_next
                PV(NGT - 1, o_ps)
                drain(NQC - 1, last=True)

    nc.compile()
    return nc


def _get_compiled():
    global _compiled
    if _compiled is None:
        _compiled = _build()
    return _compiled


def kernel(att_input: np.ndarray, Wq: np.ndarray, Wk: np.ndarray, Wv: np.ndarray) -> np.ndarray:
    nc = _get_compiled()
    in_maps = [
        {
            "x": np.ascontiguousarray(att_input[b], dtype=np.float32),
            "wq": np.ascontiguousarray(Wq, dtype=np.float32),
            "wk": np.ascontiguousarray(Wk, dtype=np.float32),
            "wv": np.ascontiguousarray(Wv, dtype=np.float32),
        }
        for b in range(B)
    ]
    res = run_bass_kernel_spmd(nc, in_maps, list(range(B)))
    return np.stack([res.results[b]["out"] for b in range(B)], axis=0)


# revision 8
# speedup vs baseline: 1.0281x; 1.0281x over previous
"""Bass/Trainium2 kernel for nn_Attention_Layer (B=8, N=4096, D=128).

Sharding: data-parallel over batch B across the 8 NeuronCores (one batch
element per core); the 128x128 Q/K/V weights are replicated.

Per-core algorithm (X = att_input[b], [4096, 128] fp32):
  1. Setup: X loaded via 4 parallel DMA queues.  PE-transposes X
     (quad-batched into PSUM); ACT evacuates+converts to fp16 xt.
     K/Q projections per 512-chunk (fp16 matmuls) evacuated to fp16
     kt/qt on DVE.  V = Xt.T @ WvT (fp16) evacuated to bf16 vext pairs
     (ones column accumulates the softmax denominator), alternating
     ACT/DVE.
  2. Main loop over 128 groups (2 k-tiles x 512 q):
       iteration g emits: S(g+1) [2 fp16 512-row matmuls, 216 ns each],
       then PV(g-1) [8 bf16 129-row matmuls, 57 ns back-to-back], then
       exp(g) [one 1024-wide ACT instruction, ~1010 ns].
     S runs one group ahead so it always completes during exp(g); the
     ACT engine never waits.  PE work/group (~950 ns) < exp (~1010 ns).
  3. Per chunk: DVE-drain O, reciprocal of ones-column sums, normalize,
     one 256KB DMA out.

dtypes: fp16 for X^T/W/Q/K (5x error margin vs bf16), bf16 for P and V
(P needs bf16 range: unnormalized exp reaches ~3.6e9), fp32 PSUM accum.
softmax max-subtraction is skipped: scores have std ~3.8, max ~22.
PSUM: S groups 2x2 banks (double buffered) + O 4 banks (129 fp32 each).
"""

import sys

if "/opt/trn_rl_repo" not in sys.path:
    sys.path.insert(0, "/opt/trn_rl_repo")

import numpy as np

import concourse.bass as bass
import concourse.mybir as mybir
import concourse.tile as tile
from concourse import bacc
from concourse.bass_utils import run_bass_kernel_spmd
from concourse.masks import make_identity

B, N, D = 8, 4096, 128
P = 128                 # partitions / tile edge
NT = N // P             # 32 n-tiles (also k-tiles)
QC = 512                # q-chunk width (max moving free dim)
NQC = N // QC           # 8 q-chunks
QT = QC // P            # 4 q-tiles per chunk
TPG = 2                 # k-tiles per exp group (exp width = TPG*512)
NG = NT // TPG          # groups per chunk (16)
NGT = NQC * NG          # total groups (128)
F32 = mybir.dt.float32
FP16 = mybir.dt.float16
BF16 = mybir.dt.bfloat16
EXPF = mybir.ActivationFunctionType.Exp

_compiled = None


def _build():
    nc = bacc.Bacc("TRN2", target_bir_lowering=False, debug=False)
    x_d = nc.dram_tensor("x", [N, D], F32, kind="ExternalInput")
    wq_d = nc.dram_tensor("wq", [D, D], F32, kind="ExternalInput")
    wk_d = nc.dram_tensor("wk", [D, D], F32, kind="ExternalInput")
    wv_d = nc.dram_tensor("wv", [D, D], F32, kind="ExternalInput")
    out_d = nc.dram_tensor("out", [N, D], F32, kind="ExternalOutput")
    out_r = out_d.rearrange("(t p) d -> p t d", p=P)

    with tile.TileContext(nc) as tc:
        with (
            tc.tile_pool(name="singles", bufs=1) as singles,
            tc.tile_pool(name="stage", bufs=2) as stage,
            tc.tile_pool(name="ptp", bufs=4) as ptp,
            tc.tile_pool(name="outp", bufs=2) as outp,
        ):
            ident = singles.tile([P, P], F32)
            make_identity(nc, ident)
            zbias = singles.tile([P, 1], F32)
            nc.vector.memset(zbias, 0.0)

            # preload the exp table while DMAs stream in
            scratch = singles.tile([P, 1], F32)
            nc.scalar.activation(scratch, zbias, EXPF, bias=zbias)

            # ---- load weights + X across 3 DMA queues; X group 0 heads the
            # sync queue so the transpose pipeline starts ASAP ----
            dma_engs = [nc.sync, nc.gpsimd, nc.scalar]
            xn = singles.tile([P, NT, D], F32)
            x_r = x_d.rearrange("(t p) d -> p t d", p=P)
            nc.sync.dma_start(
                out=xn[:, 0:QT, :], in_=x_r[:, 0:QT, :]
            )
            w_sb = {}
            for i, (name, wd) in enumerate(
                (("wq", wq_d), ("wk", wk_d), ("wv", wv_d))
            ):
                t = stage.tile([P, P], F32, tag="wload", name=f"{name}_nat")
                dma_engs[(i + 1) % 3].dma_start(out=t, in_=wd[:, :])
                w_sb[name] = t
            for g in range(1, NQC):
                dma_engs[g % 3].dma_start(
                    out=xn[:, QT * g : QT * (g + 1), :],
                    in_=x_r[:, QT * g : QT * (g + 1), :],
                )

            xt = singles.tile([P, NT, P], FP16)
            qt = [None] * NQC
            kt = [None] * NQC
            # vext pairs: [P, 2, P+1] bf16, ones in col P
            vps_sb = [
                singles.tile([P, 2, P + 1], BF16, name=f"vx{i}") for i in range(NT // 2)
            ]
            for i in range(NT // 2):
                nc.gpsimd.memset(vps_sb[i][:, :, P : P + 1], 1.0)

            # ---- setup phase (own PSUM pool, released before main loop) ----
            wmrhs = singles.tile([P, QC], F32)
            nc.vector.memset(wmrhs, 0.0)
            with tc.tile_pool(name="stage_ps", bufs=2, space="PSUM") as sps:
                # PE warmup: ~3.5us of dummy fp32 matmuls ramp the PE clock
                # to full speed while the X DMAs are still in flight
                for _ in range(2):
                    wm = sps.tile([P, QC], F32, tag="pps", bufs=3, name="warm_ps")
                    nc.tensor.matmul(wm, lhsT=ident, rhs=wmrhs, start=True, stop=True)
                # weight transposes -> [d, e] fp16
                wT = {}
                for name in ("wq", "wk", "wv"):
                    ps = sps.tile([P, P], F32, tag="wtps", bufs=1, name=f"{name}T_ps")
                    nc.tensor.transpose(ps, w_sb[name], ident)
                    t = singles.tile([P, P], FP16, name=f"{name}T")
                    nc.vector.tensor_copy(t, ps)
                    wT[name] = t

                def _proj(dst, w, nm, c):
                    pps = sps.tile([P, QC], F32, tag="pps", bufs=3, name="proj_ps")
                    nc.tensor.matmul(
                        pps,
                        lhsT=w,
                        rhs=xt[:, QT * c : QT * (c + 1), :],
                        start=True,
                        stop=True,
                    )
                    d_ = singles.tile([P, QC], FP16, tag=f"{nm}{c}", name=f"{nm}{c}")
                    nc.vector.tensor_copy(d_, pps)
                    dst[c] = d_

                # per 4-tile load group: quad transposes (ACT evacuates),
                # kt+qt projections (DVE evacuates), then the previous
                # group's V pairs (keeps the PE stream dense while this
                # group's X tiles are still in DMA flight)
                def _vpair(pair):
                    vps = sps.tile([P, 2, P], F32, tag="vps", name="v_ps")
                    nc.tensor.matmul(
                        vps[:, 0, :], lhsT=xt[:, 2 * pair, :], rhs=wT["wv"],
                        start=True, stop=True,
                    )
                    nc.tensor.matmul(
                        vps[:, 1, :], lhsT=xt[:, 2 * pair + 1, :], rhs=wT["wv"],
                        start=True, stop=True,
                    )
                    if pair % 2 == 0:
                        nc.scalar.copy(vps_sb[pair][:, :, 0:P], vps)
                    else:
                        nc.vector.tensor_copy(vps_sb[pair][:, :, 0:P], vps)

                for g in range(NQC):
                    tps = sps.tile([P, QT, P], F32, tag="tps", name="xt_ps")
                    for i in range(QT):
                        nc.tensor.transpose(tps[:, i, :], xn[:, QT * g + i, :], ident)
                    nc.scalar.copy(xt[:, QT * g : QT * (g + 1), :], tps)
                    _proj(kt, wT["wk"], "kt", g)
                    _proj(qt, wT["wq"], "qt", g)
                    if g > 0:
                        _vpair(2 * g - 2)
                        _vpair(2 * g - 1)
                _vpair(NT // 2 - 2)
                _vpair(NT // 2 - 1)

            # ---- main attention loop ----
            with (
                tc.tile_pool(name="spsum", bufs=2, space="PSUM") as spsum,
                tc.tile_pool(name="opsum", bufs=1, space="PSUM") as opsum,
            ):
                def S_group(gg):
                    c, g = divmod(gg, NG)
                    sg = spsum.tile([P, TPG, QC], F32, tag="sg", name="s_ps")
                    for i in range(TPG):
                        t = TPG * g + i
                        nc.tensor.matmul(
                            sg[:, i, :],
                            lhsT=kt[t // QT][:, (t % QT) * P : (t % QT + 1) * P],
                            rhs=qt[c],
                            start=True,
                            stop=True,
                        )
                    return sg

                o_ps = None

                def PV(gg, o_ps):
                    g = gg % NG
                    pt = pts[gg % 4]
                    for i in range(TPG):
                        tp = TPG * g + i
                        for j in range(QT):
                            nc.tensor.matmul(
                                o_ps[j],
                                lhsT=pt[:, i, j * P : (j + 1) * P],
                                rhs=vps_sb[tp // 2][:, tp % 2, :],
                                start=(tp == 0),
                                stop=(tp == NT - 1),
                                skip_group_check=True,
                            )

                def drain(c, last=False):
                    # On the last chunk ACT is idle: split the drain across
                    # ACT and DVE to shorten the tail.
                    oc = outp.tile([P, QT, P + 1], F32, tag="oc", name="oc")
                    for j in range(QT):
                        if last and j < 2:
                            nc.scalar.copy(oc[:, j, :], o_ps[j])
                        else:
                            nc.vector.tensor_copy(oc[:, j, :], o_ps[j])
                    ot = outp.tile([P, QT, P], F32, tag="ot", name="ot")
                    for j in range(QT):
                        rinv = outp.tile([P, 1], F32, tag="rinv", name="rinv")
                        nc.vector.reciprocal(rinv, oc[:, j, P : P + 1])
                        if last and j < 2:
                            nc.scalar.activation(
                                ot[:, j, :], oc[:, j, 0:P],
                                mybir.ActivationFunctionType.Copy,
                                bias=0.0, scale=rinv[:, 0:1],
                            )
                        else:
                            nc.vector.tensor_scalar_mul(
                                ot[:, j, :], oc[:, j, 0:P], rinv[:, 0:1]
                            )
                    for j in range(QT):
                        eng = dma_engs[j % 3] if last else dma_engs[j % 2]
                        eng.dma_start(
                            out=out_r[:, QT * c + j, :], in_=ot[:, j, :]
                        )

                pts = [None] * 4
                sg_cur = S_group(0)
                for gg in range(NGT):
                    sg_next = S_group(gg + 1) if gg < NGT - 1 else None
                    if gg % NG == 1:
                        # first PV of a chunk: allocate fresh O accumulators
                        o_ps = [
                            opsum.tile([P, P + 1], F32, tag=f"o{j}", name=f"o{j}")
                            for j in range(QT)
                        ]
                    if gg > 0:
                        PV(gg - 1, o_ps)
                        if (gg - 1) % NG == NG - 1:
                            drain((gg - 1) // NG)
                    pt = ptp.tile([P, TPG, QC], BF16, tag="pt", name="pt")
                    nc.scalar.activation(pt, sg_cur, EXPF, bias=zbias)
                    pts[gg % 4] = pt
                    sg_cur = sg_next
                PV(NGT - 1, o_ps)
                drain(NQC - 1, last=True)

    nc.compile()
    return nc


def _get_compiled():
    global _compiled
    if _compiled is None:
        _compiled = _build()
    return _compiled


def kernel(att_input: np.ndarray, Wq: np.ndarray, Wk: np.ndarray, Wv: np.ndarray) -> np.ndarray:
    nc = _get_compiled()
    in_maps = [
        {
            "x": np.ascontiguousarray(att_input[b], dtype=np.float32),
            "wq": np.ascontiguousarray(Wq, dtype=np.float32),
            "wk": np.ascontiguousarray(Wk, dtype=np.float32),
            "wv": np.ascontiguousarray(Wv, dtype=np.float32),
        }
        for b in range(B)
    ]
    res = run_bass_kernel_spmd(nc, in_maps, list(range(B)))
    return np.stack([res.results[b]["out"] for b in range(B)], axis=0)
